# revision 15
# baseline (speedup 1.0000x reference)
"""Trainium2 Bass kernel for GroundTruthBasedPriorNetwork.

Per-node tiny MLP over a banded DAG, batched over 131072 samples:
    x[b, n, p]  = gt_labels[b, parent_idx[n, p]]          (N=64 nodes, P=8)
    h[b, n, :]  = tanh(W1[n] @ x[b, n, :] + b1[n])        (HID=16)
    mus[b, n]   = W2[n] . h[b, n, :] + b2[n]
    logvars     = zeros

Strategy: pure data parallel over 8 NeuronCores (batch split 8x16384).
The parent gather is folded on the host into per-node-block dense weight
matrices (bias folded in via a ones-row), all bf16.

The PE on these cores is clock-gated to 1.2 GHz (the HAM never raises it
to 2.4 even under sustained matmul load), so matmul cost is pure
column-streaming at 1.2 GHz and the kernel minimizes streamed columns:

  L1: the DAG band means hidden block t (nodes 8t..8t+8, 128 padded
  hidden units) depends on only <=15 gt rows + a ones row.  Each block's
  inputs fit a 32-partition strip (two blocks share a strip), so four
  K=32 matmuls run CONCURRENTLY in the 128x128 array via tile_position
  row tiling, each writing a different PSUM bank.  A group's 8 blocks
  take 2 waves (~4x fewer PE-busy cycles than 8 serial K=65 matmuls).

  Tanh on ScalarE (the other near-saturated engine, ~2.0us/group) reads
  the (128 x 2048) PSUM l1 tile and writes bf16 h to SBUF.

  L2: 8 accumulating K=128 matmuls against a block-structured W2T into a
  (64 x 256) corner of the same l1 tile (dead after the Tanh read) so l1
  can double-buffer across all 8 PSUM banks.  A 1-element DVE memset is
  emitted as each fresh l1 tile's first toucher so the pool-rotation
  dependency (on the previous evac) is carried by the idle DVE instead
  of stalling the PE queue.  A DVE tensor_scalar add of b2 evacuates
  mus to SBUF in bf16; output is un-transposed / cast on the host.
"""

import os

import numpy as np

NUM_NODES = 64
MAX_P = 8
HID = 16
HFULL = NUM_NODES * HID  # 1024
BATCH = 131072
NCORES = 8
BC = BATCH // NCORES  # 16384 per core
GROUP = 256  # batch columns per group
NG = BC // GROUP  # 64 groups
OUT_CHUNK = 16  # groups per output DMA chunk
NBLK = 8  # hidden blocks (8 nodes x 16 hid = 128 each)

_COMPILED = {}


def _bf16(a):
    import ml_dtypes

    return np.asarray(a, np.float32).astype(ml_dtypes.bfloat16)


def _build_weights(W1, b1, W2, b2, parent_idx):
    """Host-side preprocessing: returns (wpack bf16, b2 f32, sel_idx).

    wpack cols [0,1024): W1R -- strip-packed per-block first-layer weights.
      Block t (hidden cols 128t..128t+128) lives in partition strip
      s = t//2 (rows 32s..32s+32) at offset 0 (t even) or after block
      t-1's rows (t odd); its rows are the block's active gt rows plus a
      ones/bias row.
    wpack cols [1024,1536): W2T block-structured second layer (as before).
    sel_idx (128,): which conceptual xt row (0-63 gt.T, 64 ones, 65 zero)
      each on-device xt partition holds.
    """
    W1 = np.asarray(W1, np.float32)
    b1 = np.asarray(b1, np.float32)
    W2 = np.asarray(W2, np.float32)
    b2 = np.asarray(b2, np.float32)
    parent_idx = np.asarray(parent_idx)

    # Dense gather-folded layer 1: w1_full[j, 16n+h] then bias row.
    w1_full = np.zeros((NUM_NODES, HFULL), np.float32)
    for n in range(NUM_NODES):
        for p in range(MAX_P):
            j = int(parent_idx[n, p])
            w1_full[j, 16 * n : 16 * n + 16] += W1[n, :, p]
    w1_aug = np.concatenate([w1_full, b1.reshape(1, HFULL)], axis=0)  # (65, 1024)

    # Strip packing: block t -> (strip t//2, row offset), K=32 per strip.
    w1r = np.zeros((128, HFULL), np.float32)
    sel_idx = np.full(128, 65, np.int64)  # default: zero row
    for s in range(4):
        off = 0
        for t in (2 * s, 2 * s + 1):
            cols = slice(128 * t, 128 * (t + 1))
            rows = [j for j in range(NUM_NODES) if np.any(w1_aug[j, cols])]
            rows.append(NUM_NODES)  # bias row (multiplies the ones entry)
            assert off + len(rows) <= 32, (s, t, off, len(rows))
            for k, j in enumerate(rows):
                w1r[32 * s + off + k, cols] = w1_aug[j, cols]
                sel_idx[32 * s + off + k] = j if j < NUM_NODES else NUM_NODES
            off += len(rows)

    # W2T[p, 64t+n] = W2[n, hf%16] where hf = 128t+p and n == hf//16, else 0
    w2t = np.zeros((128, 8 * NUM_NODES), np.float32)
    for t in range(8):
        for p in range(128):
            hf = 128 * t + p
            n = hf // HID
            w2t[p, NUM_NODES * t + n] = W2[n, hf % HID]

    wpack = np.zeros((128, HFULL + 8 * NUM_NODES), np.float32)
    wpack[:, :HFULL] = w1r
    wpack[:, HFULL : HFULL + 8 * NUM_NODES] = w2t
    return _bf16(wpack), np.ascontiguousarray(b2.reshape(NUM_NODES, 1)), sel_idx


def _build_nc():
    import concourse.bacc as bacc
    import concourse.mybir as mybir
    import concourse.tile as tile
    from contextlib import ExitStack

    f32 = mybir.dt.float32
    bf16 = mybir.dt.bfloat16

    nc = bacc.Bacc("TRN2", target_bir_lowering=False, debug=False,
                   num_devices=NCORES)

    CW = HFULL + 8 * NUM_NODES  # 1536
    xt_d = nc.dram_tensor("xt", [128, BC], bf16, kind="ExternalInput")
    wpack_d = nc.dram_tensor("wpack", [128, CW], bf16, kind="ExternalInput")
    b2_d = nc.dram_tensor("b2", [NUM_NODES, 1], f32, kind="ExternalInput")
    out_d = nc.dram_tensor("out", [NUM_NODES, BC], bf16, kind="ExternalOutput")

    # xt chunk column ranges: a small first chunk so the first L1 matmul
    # can start ~2us earlier; the rest loaded while compute runs.
    XT_EDGES = [0, 1024, 6144, 11264, BC]
    NOC = NG // OUT_CHUNK  # 4 output chunks
    OCW = OUT_CHUNK * GROUP  # 4096

    with tile.TileContext(nc) as tc, ExitStack() as ctx:
        consts = ctx.enter_context(tc.tile_pool(name="consts", bufs=1))
        xt_pool = ctx.enter_context(tc.tile_pool(name="xt", bufs=1))
        out_pool = ctx.enter_context(tc.tile_pool(name="out", bufs=1))
        h_pool = ctx.enter_context(tc.tile_pool(name="h", bufs=2))
        l1_pool = ctx.enter_context(tc.tile_pool(name="l1", bufs=2, space="PSUM"))

        wpack_sb = consts.tile([128, CW], bf16, tag="wpack")
        b2_sb = consts.tile([NUM_NODES, 1], f32, tag="b2")
        nc.sync.dma_start(wpack_sb[:], wpack_d.ap())
        nc.sync.dma_start(b2_sb[:], b2_d.ap())
        w1_sb = wpack_sb[:, :HFULL]
        w2t_sb = wpack_sb[:, HFULL : HFULL + 8 * NUM_NODES]

        xt_tiles = []
        for k in range(len(XT_EDGES) - 1):
            xt_sb = xt_pool.tile(
                [128, XT_EDGES[k + 1] - XT_EDGES[k]], bf16,
                tag=f"xt{k}", name=f"xt_sb{k}",
            )
            xt_tiles.append(xt_sb)

        scratch = consts.tile([1, 2], bf16, tag="scratch")

        # Load the first batch chunk, then fence: the barrier absorbs the
        # const + first-chunk DMA waits so the matmuls (whose weight-load
        # micro-op has a tight sync-wait budget) carry at most a couple of
        # semaphore waits each.  The dummy Tanh (on a DMA-independent
        # memset scratch, so it can issue the moment the preamble ends)
        # pulls the ~2.7us activation table load under the DMA wait
        # instead of stalling the first real group.
        nc.vector.memset(scratch[:, 0:1], 0.0)
        nc.scalar.activation(
            scratch[:, 1:2], scratch[:, 0:1],
            mybir.ActivationFunctionType.Tanh,
        )
        nc.sync.dma_start(xt_tiles[0][:], xt_d.ap()[:, 0 : XT_EDGES[1]])
        tc.strict_bb_all_engine_barrier()
        for k in range(1, len(xt_tiles)):
            nc.sync.dma_start(
                xt_tiles[k][:], xt_d.ap()[:, XT_EDGES[k] : XT_EDGES[k + 1]]
            )

        out_tiles = [
            out_pool.tile([NUM_NODES, OCW], bf16, tag=f"out{k}", name=f"out_sb{k}")
            for k in range(NOC)
        ]

        l1_tiles = [None] * NG
        h_tiles = [None] * NG

        def emit_l1(g):
            col = g * GROUP
            xk = next(
                k for k in range(len(xt_tiles)) if XT_EDGES[k + 1] > col
            )
            xoff = col - XT_EDGES[xk]
            l1 = l1_pool.tile([128, 8 * GROUP], f32, tag="l1")
            l1_tiles[g] = l1
            # Two waves of four concurrent row-tiled K=32 matmuls; wave w
            # covers blocks t = 2s+w, hitting PSUM banks 0..3 once each.
            for w in (0, 1):
                for s in range(4):
                    t = 2 * s + w
                    nc.tensor.matmul(
                        l1[:, t * GROUP : (t + 1) * GROUP],
                        w1_sb[32 * s : 32 * s + 32, t * 128 : (t + 1) * 128],
                        xt_tiles[xk][32 * s : 32 * s + 32, xoff : xoff + GROUP],
                        start=True,
                        stop=True,
                        tile_position=(32 * s, 0),
                    )

        def emit_act(g):
            h = h_pool.tile([128, 8 * GROUP], bf16, tag="h")
            h_tiles[g] = h
            nc.scalar.activation(
                h[:], l1_tiles[g][:], mybir.ActivationFunctionType.Tanh
            )

        def emit_l2(g):
            # mus reuses the first PSUM bank of this group's (now consumed)
            # l1 tile, so l1 can double-buffer across all 8 PSUM banks.
            h = h_tiles[g]
            mus = l1_tiles[g][:NUM_NODES, :GROUP]
            # Two concurrent col-tiled accumulation chains: strip j computes
            # nodes [32j, 32j+32), which (block-diagonal W2) only need
            # hidden blocks 4j..4j+4 -- so each chain is 4 matmuls and the
            # two chains stream their rhs concurrently.
            for k in range(4):
                for j in (0, 1):
                    t = 4 * j + k
                    nc.tensor.matmul(
                        mus[32 * j : 32 * j + 32, :],
                        w2t_sb[:, 64 * t + 32 * j : 64 * t + 32 * j + 32],
                        h[:, t * GROUP : (t + 1) * GROUP],
                        start=(k == 0),
                        stop=(k == 3),
                        tile_position=(0, 32 * j),
                    )
            ok, ooff = divmod(g * GROUP, OCW)
            nc.vector.tensor_scalar_add(
                out_tiles[ok][:, ooff : ooff + GROUP], mus, b2_sb[:]
            )
            h_tiles[g] = None
            l1_tiles[g] = None
            if (g + 1) % OUT_CHUNK == 0:
                k = g // OUT_CHUNK
                nc.sync.dma_start(
                    out_d.ap()[:, k * OCW : (k + 1) * OCW], out_tiles[k][:]
                )

        # PE program order: ... L2(g-1), L1(g+1), L2(g) ... -- L1 runs two
        # groups ahead of L2, so the l1-buffer-reuse chain (L2(g) -> mus
        # evac TS(g) -> L1(g+2)) spans two Tanh periods and nearly fits
        # inside them; the pipeline is close to Tanh-bound.
        emit_l1(0)
        emit_act(0)
        emit_l1(1)
        emit_act(1)
        for g in range(NG - 2):
            emit_l2(g)
            emit_l1(g + 2)
            emit_act(g + 2)
        emit_l2(NG - 2)
        emit_l2(NG - 1)

    nc.finalize()
    return nc


def _get_nc():
    if "nc" not in _COMPILED:
        _COMPILED["nc"] = _build_nc()
    return _COMPILED["nc"]


def kernel(gt_labels, W1, b1, W2, b2, parent_idx):
    import ml_dtypes
    from concourse.bass_utils import run_bass_kernel_spmd

    gt_labels = np.asarray(gt_labels, np.float32)
    wpack, b2c, sel_idx = _build_weights(W1, b1, W2, b2, parent_idx)

    in_maps = []
    for c in range(NCORES):
        xc = gt_labels[c * BC : (c + 1) * BC]  # (16384, 64)
        xt66 = np.empty((NUM_NODES + 2, BC), np.float32)
        xt66[:NUM_NODES] = xc.T
        xt66[NUM_NODES] = 1.0
        xt66[NUM_NODES + 1] = 0.0
        xtr = _bf16(xt66[sel_idx])  # (128, BC) strip-packed
        in_maps.append({"xt": xtr, "wpack": wpack, "b2": b2c})

    nc = _get_nc()
    trace = bool(int(os.environ.get("KERNEL_TRACE", "0")))
    res = run_bass_kernel_spmd(nc, in_maps, list(range(NCORES)), trace=trace)
    if trace and res.exec_time_ns is not None:
        print(f"HW exec time: {res.exec_time_ns} ns")
        _COMPILED["exec_time_ns"] = res.exec_time_ns

    mus = np.empty((BATCH, NUM_NODES), np.float32)
    for c in range(NCORES):
        mus[c * BC : (c + 1) * BC] = res.results[c]["out"].astype(np.float32).T
    mus = mus.reshape(BATCH, NUM_NODES, 1)
    logvars = np.zeros_like(mus)
    return mus, logvars


# revision 16
# speedup vs baseline: 1.0281x; 1.0281x over previous
"""Trainium2 Bass kernel for GroundTruthBasedPriorNetwork.

Per-node tiny MLP over a banded DAG, batched over 131072 samples:
    x[b, n, p]  = gt_labels[b, parent_idx[n, p]]          (N=64 nodes, P=8)
    h[b, n, :]  = tanh(W1[n] @ x[b, n, :] + b1[n])        (HID=16)
    mus[b, n]   = W2[n] . h[b, n, :] + b2[n]
    logvars     = zeros

Strategy: pure data parallel over 8 NeuronCores (batch split 8x16384).
The parent gather is folded on the host into per-node-block dense weight
matrices (bias folded in via a ones-row), all bf16.

The PE on these cores is clock-gated to 1.2 GHz (the HAM never raises it
to 2.4 even under sustained matmul load), so matmul cost is pure
column-streaming at 1.2 GHz and the kernel minimizes streamed columns:

  L1: the DAG band means hidden block t (nodes 8t..8t+8, 128 padded
  hidden units) depends on only <=15 gt rows + a ones row.  Each block's
  inputs fit a 32-partition strip (two blocks share a strip), so four
  K=32 matmuls run CONCURRENTLY in the 128x128 array via tile_position
  row tiling, each writing a different PSUM bank.  A group's 8 blocks
  take 2 waves (~4x fewer PE-busy cycles than 8 serial K=65 matmuls).

  Tanh on ScalarE (the other near-saturated engine, ~2.0us/group) reads
  the (128 x 2048) PSUM l1 tile and writes bf16 h to SBUF.

  L2: 8 accumulating K=128 matmuls against a block-structured W2T into a
  (64 x 256) corner of the same l1 tile (dead after the Tanh read) so l1
  can double-buffer across all 8 PSUM banks.  A 1-element DVE memset is
  emitted as each fresh l1 tile's first toucher so the pool-rotation
  dependency (on the previous evac) is carried by the idle DVE instead
  of stalling the PE queue.  A DVE tensor_scalar add of b2 evacuates
  mus to SBUF in bf16; output is un-transposed / cast on the host.
"""

import os

import numpy as np

NUM_NODES = 64
MAX_P = 8
HID = 16
HFULL = NUM_NODES * HID  # 1024
BATCH = 131072
NCORES = 8
BC = BATCH // NCORES  # 16384 per core
GROUP = 256  # batch columns per group
NG = BC // GROUP  # 64 groups
OUT_CHUNK = 8  # groups per output DMA chunk
NBLK = 8  # hidden blocks (8 nodes x 16 hid = 128 each)

_COMPILED = {}


def _bf16(a):
    import ml_dtypes

    return np.asarray(a, np.float32).astype(ml_dtypes.bfloat16)


def _build_weights(W1, b1, W2, b2, parent_idx):
    """Host-side preprocessing: returns (wpack bf16, b2 f32, sel_idx).

    wpack cols [0,1024): W1R -- strip-packed per-block first-layer weights.
      Block t (hidden cols 128t..128t+128) lives in partition strip
      s = t//2 (rows 32s..32s+32) at offset 0 (t even) or after block
      t-1's rows (t odd); its rows are the block's active gt rows plus a
      ones/bias row.
    wpack cols [1024,1536): W2T block-structured second layer (as before).
    sel_idx (128,): which conceptual xt row (0-63 gt.T, 64 ones, 65 zero)
      each on-device xt partition holds.
    """
    W1 = np.asarray(W1, np.float32)
    b1 = np.asarray(b1, np.float32)
    W2 = np.asarray(W2, np.float32)
    b2 = np.asarray(b2, np.float32)
    parent_idx = np.asarray(parent_idx)

    # Dense gather-folded layer 1: w1_full[j, 16n+h] then bias row.
    w1_full = np.zeros((NUM_NODES, HFULL), np.float32)
    for n in range(NUM_NODES):
        for p in range(MAX_P):
            j = int(parent_idx[n, p])
            w1_full[j, 16 * n : 16 * n + 16] += W1[n, :, p]
    w1_aug = np.concatenate([w1_full, b1.reshape(1, HFULL)], axis=0)  # (65, 1024)

    # Strip packing: block t -> (strip t//2, row offset), K=32 per strip.
    w1r = np.zeros((128, HFULL), np.float32)
    sel_idx = np.full(128, 65, np.int64)  # default: zero row
    for s in range(4):
        off = 0
        for t in (2 * s, 2 * s + 1):
            cols = slice(128 * t, 128 * (t + 1))
            rows = [j for j in range(NUM_NODES) if np.any(w1_aug[j, cols])]
            rows.append(NUM_NODES)  # bias row (multiplies the ones entry)
            assert off + len(rows) <= 32, (s, t, off, len(rows))
            for k, j in enumerate(rows):
                w1r[32 * s + off + k, cols] = w1_aug[j, cols]
                sel_idx[32 * s + off + k] = j if j < NUM_NODES else NUM_NODES
            off += len(rows)

    # W2T[p, 64t+n] = W2[n, hf%16] where hf = 128t+p and n == hf//16, else 0
    w2t = np.zeros((128, 8 * NUM_NODES), np.float32)
    for t in range(8):
        for p in range(128):
            hf = 128 * t + p
            n = hf // HID
            w2t[p, NUM_NODES * t + n] = W2[n, hf % HID]

    return (
        _bf16(w1r),
        _bf16(w2t),
        np.ascontiguousarray(b2.reshape(NUM_NODES, 1)),
        sel_idx,
    )


def _build_nc():
    import concourse.bacc as bacc
    import concourse.mybir as mybir
    import concourse.tile as tile
    from contextlib import ExitStack

    f32 = mybir.dt.float32
    bf16 = mybir.dt.bfloat16

    nc = bacc.Bacc("TRN2", target_bir_lowering=False, debug=False,
                   num_devices=NCORES)

    xt_d = nc.dram_tensor("xt", [128, BC], bf16, kind="ExternalInput")
    w1r_d = nc.dram_tensor("w1r", [128, HFULL], bf16, kind="ExternalInput")
    w2t_d = nc.dram_tensor("w2t", [128, 8 * NUM_NODES], bf16, kind="ExternalInput")
    b2_d = nc.dram_tensor("b2", [NUM_NODES, 1], f32, kind="ExternalInput")
    out_d = nc.dram_tensor("out", [NUM_NODES, BC], bf16, kind="ExternalOutput")

    # xt chunk column ranges: a small first chunk so the first L1 matmul
    # can start ~2us earlier; the rest loaded while compute runs.
    XT_EDGES = [0, 1024, 6144, 11264, BC]
    NOC = NG // OUT_CHUNK  # 4 output chunks
    OCW = OUT_CHUNK * GROUP  # 4096

    with tile.TileContext(nc) as tc, ExitStack() as ctx:
        consts = ctx.enter_context(tc.tile_pool(name="consts", bufs=1))
        xt_pool = ctx.enter_context(tc.tile_pool(name="xt", bufs=1))
        out_pool = ctx.enter_context(tc.tile_pool(name="out", bufs=1))
        h_pool = ctx.enter_context(tc.tile_pool(name="h", bufs=2))
        l1_pool = ctx.enter_context(tc.tile_pool(name="l1", bufs=2, space="PSUM"))

        w1_sb = consts.tile([128, HFULL], bf16, tag="w1r")
        w2t_sb = consts.tile([128, 8 * NUM_NODES], bf16, tag="w2t")
        b2_sb = consts.tile([NUM_NODES, 1], f32, tag="b2")
        nc.sync.dma_start(w1_sb[:], w1r_d.ap())

        xt_tiles = []
        for k in range(len(XT_EDGES) - 1):
            xt_sb = xt_pool.tile(
                [128, XT_EDGES[k + 1] - XT_EDGES[k]], bf16,
                tag=f"xt{k}", name=f"xt_sb{k}",
            )
            xt_tiles.append(xt_sb)

        scratch = consts.tile([1, 2], bf16, tag="scratch")

        # Load the first batch chunk, then fence: the barrier absorbs the
        # const + first-chunk DMA waits so the matmuls (whose weight-load
        # micro-op has a tight sync-wait budget) carry at most a couple of
        # semaphore waits each.  The dummy Tanh (on a DMA-independent
        # memset scratch, so it can issue the moment the preamble ends)
        # pulls the ~2.7us activation table load under the DMA wait
        # instead of stalling the first real group.
        nc.vector.memset(scratch[:, 0:1], 0.0)
        nc.scalar.activation(
            scratch[:, 1:2], scratch[:, 0:1],
            mybir.ActivationFunctionType.Tanh,
        )
        nc.sync.dma_start(xt_tiles[0][:], xt_d.ap()[:, 0 : XT_EDGES[1]])
        tc.strict_bb_all_engine_barrier()
        # Not needed until the first L2, ~2 periods after the barrier.
        nc.sync.dma_start(w2t_sb[:], w2t_d.ap())
        nc.sync.dma_start(b2_sb[:], b2_d.ap())
        for k in range(1, len(xt_tiles)):
            nc.sync.dma_start(
                xt_tiles[k][:], xt_d.ap()[:, XT_EDGES[k] : XT_EDGES[k + 1]]
            )

        out_tiles = [
            out_pool.tile([NUM_NODES, OCW], bf16, tag=f"out{k}", name=f"out_sb{k}")
            for k in range(NOC)
        ]

        l1_tiles = [None] * NG
        h_tiles = [None] * NG

        def emit_l1(g):
            col = g * GROUP
            xk = next(
                k for k in range(len(xt_tiles)) if XT_EDGES[k + 1] > col
            )
            xoff = col - XT_EDGES[xk]
            l1 = l1_pool.tile([128, 8 * GROUP], f32, tag="l1")
            l1_tiles[g] = l1
            # Two waves of four concurrent row-tiled K=32 matmuls; wave w
            # covers blocks t = 2s+w, hitting PSUM banks 0..3 once each.
            for w in (0, 1):
                for s in range(4):
                    t = 2 * s + w
                    nc.tensor.matmul(
                        l1[:, t * GROUP : (t + 1) * GROUP],
                        w1_sb[32 * s : 32 * s + 32, t * 128 : (t + 1) * 128],
                        xt_tiles[xk][32 * s : 32 * s + 32, xoff : xoff + GROUP],
                        start=True,
                        stop=True,
                        tile_position=(32 * s, 0),
                    )

        def emit_act(g):
            h = h_pool.tile([128, 8 * GROUP], bf16, tag="h")
            h_tiles[g] = h
            nc.scalar.activation(
                h[:], l1_tiles[g][:], mybir.ActivationFunctionType.Tanh
            )

        def emit_l2(g):
            # mus reuses the first PSUM bank of this group's (now consumed)
            # l1 tile, so l1 can double-buffer across all 8 PSUM banks.
            h = h_tiles[g]
            mus = l1_tiles[g][:NUM_NODES, :GROUP]
            # Two concurrent col-tiled accumulation chains: strip j computes
            # nodes [32j, 32j+32), which (block-diagonal W2) only need
            # hidden blocks 4j..4j+4 -- so each chain is 4 matmuls and the
            # two chains stream their rhs concurrently.
            for k in range(4):
                for j in (0, 1):
                    t = 4 * j + k
                    nc.tensor.matmul(
                        mus[32 * j : 32 * j + 32, :],
                        w2t_sb[:, 64 * t + 32 * j : 64 * t + 32 * j + 32],
                        h[:, t * GROUP : (t + 1) * GROUP],
                        start=(k == 0),
                        stop=(k == 3),
                        tile_position=(0, 32 * j),
                    )
            ok, ooff = divmod(g * GROUP, OCW)
            nc.vector.tensor_scalar_add(
                out_tiles[ok][:, ooff : ooff + GROUP], mus, b2_sb[:]
            )
            h_tiles[g] = None
            l1_tiles[g] = None
            if (g + 1) % OUT_CHUNK == 0:
                k = g // OUT_CHUNK
                nc.sync.dma_start(
                    out_d.ap()[:, k * OCW : (k + 1) * OCW], out_tiles[k][:]
                )

        # PE program order: ... L2(g-1), L1(g+1), L2(g) ... -- L1 runs two
        # groups ahead of L2, so the l1-buffer-reuse chain (L2(g) -> mus
        # evac TS(g) -> L1(g+2)) spans two Tanh periods and nearly fits
        # inside them; the pipeline is close to Tanh-bound.
        emit_l1(0)
        emit_act(0)
        emit_l1(1)
        emit_act(1)
        for g in range(NG - 2):
            emit_l2(g)
            emit_l1(g + 2)
            emit_act(g + 2)
        emit_l2(NG - 2)
        emit_l2(NG - 1)

    nc.finalize()
    return nc


def _get_nc():
    if "nc" not in _COMPILED:
        _COMPILED["nc"] = _build_nc()
    return _COMPILED["nc"]


def kernel(gt_labels, W1, b1, W2, b2, parent_idx):
    import ml_dtypes
    from concourse.bass_utils import run_bass_kernel_spmd

    gt_labels = np.asarray(gt_labels, np.float32)
    w1r, w2t, b2c, sel_idx = _build_weights(W1, b1, W2, b2, parent_idx)

    in_maps = []
    for c in range(NCORES):
        xc = gt_labels[c * BC : (c + 1) * BC]  # (16384, 64)
        xt66 = np.empty((NUM_NODES + 2, BC), np.float32)
        xt66[:NUM_NODES] = xc.T
        xt66[NUM_NODES] = 1.0
        xt66[NUM_NODES + 1] = 0.0
        xtr = _bf16(xt66[sel_idx])  # (128, BC) strip-packed
        in_maps.append({"xt": xtr, "w1r": w1r, "w2t": w2t, "b2": b2c})

    nc = _get_nc()
    trace = bool(int(os.environ.get("KERNEL_TRACE", "0")))
    res = run_bass_kernel_spmd(nc, in_maps, list(range(NCORES)), trace=trace)
    if trace and res.exec_time_ns is not None:
        print(f"HW exec time: {res.exec_time_ns} ns")
        _COMPILED["exec_time_ns"] = res.exec_time_ns

    mus = np.empty((BATCH, NUM_NODES), np.float32)
    for c in range(NCORES):
        mus[c * BC : (c + 1) * BC] = res.results[c]["out"].astype(np.float32).T
    mus = mus.reshape(BATCH, NUM_NODES, 1)
    logvars = np.zeros_like(mus)
    return mus, logvars
